# revision 18
# baseline (speedup 1.0000x reference)
"""Trainium2 Bass kernel for nn_GatedAttention (linear attention with sigmoid
gate).

Strategy: shard the 16384 token rows across 8 cores (2048 each; cores 2b,2b+1
hold batch b). Per core, two phases:
  A: K,V projections (token-major) + per-head kv' = K^T [V|1] accumulated in
     PSUM over all local tokens (the ones column folds k_sum into kv').
  -- pairwise AllReduce of kv' between the two cores sharing a batch --
  B: Q,G projections (feature-major), out^T = kv'^T @ Q per head (block-diag
     per head pair), normalizer z = SCALE/max(q.k_sum,eps) applied via tiny
     selector matmuls, gate, and the final output projection, feature-major.
Host transposes x to feature-major and pre-transposes weights; output returns
feature-major per-core slabs that the host transposes back.

Scheduling notes (sim-derived):
 - DMA order: wk, X[:, :512], wv, X[:, 512:], wq, wg, wo -- the first K-proj
   matmul only waits for ~3MB instead of the full ~14MB input set.
 - kv' accumulates in PSUM across all 16 m-tiles (start/stop at the ends),
   no per-mtile DVE adds.
 - ps_proj PSUM pool is allocated before phase A's pools so phase B's Q/G
   projections don't wait on a PSUM bank recycle barrier.
 - Phase B emits projections one chunk ahead of attention tails so the
   AllReduce hides behind ~54us of independent matmuls.
 - Q-elu and G-sigmoid are batched (8+8, not interleaved): exp and sigmoid
   live in different scalar-engine act tables; interleaving reloads ~1.3us.
 - y stores DMA directly from PSUM (no scalar copy).
"""
import sys

sys.path.insert(0, "/opt/trn_rl_repo")

import numpy as np
import ml_dtypes

B, N, DIM = 4, 4096, 1024
HEADS, DH = 16, 64
SCALE = DH ** -0.5
N_CORES = 8
TPC = B * N // N_CORES      # 2048 tokens per core
NMT = TPC // 128            # 16 m-tiles (phase A)
CHUNK = 512
NCH = TPC // CHUNK          # 4 chunks (phase B)
CLAMP = 1e-6 / SCALE

DT_MODE = "bf16"            # "bf16" | "f32r" | "f32"

_CACHE = {}


def _build(dt_mode=DT_MODE, reps=1):
    import concourse.bacc as bacc
    import concourse.bass as bass
    import concourse.tile as tile
    from concourse import mybir

    AF = mybir.ActivationFunctionType
    F32 = mybir.dt.float32
    DT = mybir.dt.bfloat16 if dt_mode == "bf16" else mybir.dt.float32

    def mm(ap):
        # matmul-operand view: reduced-precision f32 mode uses float32r APs
        return ap.bitcast(mybir.dt.float32r) if dt_mode == "f32r" else ap

    ts = bass.ts

    nc = bacc.Bacc("TRN2", target_bir_lowering=False, debug=False,
                   num_devices=N_CORES)
    xt = nc.dram_tensor("xt", [DIM, TPC], DT, kind="ExternalInput")
    w_in = {}
    for nm in ("wk", "wv", "wq", "wg", "wo"):
        w_in[nm] = nc.dram_tensor(nm, [DIM, DIM], DT, kind="ExternalInput")
    bg_d = nc.dram_tensor("bg", [DIM], F32, kind="ExternalInput")
    y_d = nc.dram_tensor("y", [DIM, TPC], F32, kind="ExternalOutput")
    cc_in = nc.dram_tensor("cc_in", [128, 8, 65], DT)
    cc_out = nc.dram_tensor("cc_out", [128, 8, 65], DT)

    with tile.TileContext(nc, num_cores=N_CORES) as tc:
        with (
            tc.tile_pool(name="persist", bufs=1) as persist,
            tc.tile_pool(name="pb_big", bufs=2) as pb_big,
            tc.tile_pool(name="ps_proj", bufs=2, space="PSUM") as ps_proj,
        ):
            X = persist.tile([128, 8, TPC], DT, tag="x")
            wsb = {}
            for nm in ("wq", "wg", "wo"):
                wsb[nm] = persist.tile([128, 8, DIM], DT, tag=nm, name=nm)
            bg_sb = persist.tile([128, 8], F32, tag="bg")
            sel_np = np.zeros((16, 8, 128), _np_dt(dt_mode))
            for p in range(8):
                sel_np[2 * p, p, 0:64] = 1.0
                sel_np[2 * p + 1, p, 64:128] = 1.0
            sel_d = nc.inline_tensor(sel_np, name="sel_const")
            sel = persist.tile([16, 8, 128], DT, tag="sel")

            for _rep in range(reps):
                _phases(nc, tc, bass, mybir, AF, F32, DT, mm, ts, X, wsb,
                        bg_sb, sel, sel_d, xt, bg_d, w_in, cc_in, cc_out,
                        y_d, ps_proj, pb_big, first=(_rep == 0))
    nc.compile()
    return nc


def _phases(nc, tc, bass, mybir, AF, F32, DT, mm, ts, X, wsb, bg_sb, sel,
            sel_d, xt, bg_d, w_in, cc_in, cc_out, y_d, ps_proj, pb_big,
            first):
    # ---------------- phase A ----------------
    with (
        tc.tile_pool(name="pa_w", bufs=1) as pa_w,
        tc.tile_pool(name="pa_tmp", bufs=2) as pa_tmp,
        tc.tile_pool(name="pa_ps", bufs=2, space="PSUM") as pa_ps,
        tc.tile_pool(name="kv_ps", bufs=1, space="PSUM") as kv_pool,
    ):
        for nm in ("wk", "wv"):
            wsb[nm] = pa_w.tile([128, 8, DIM], DT, tag=nm, name=nm)
        # DMA issue order gates PE start: wk then the first 512 token
        # columns of X unblock the first K-proj after ~3MB of traffic.
        for i in range(8):
            nc.sync.dma_start(out=wsb["wk"][:, i, :],
                              in_=w_in["wk"].ap()[ts(i, 128), :])
        if first:
            for i in range(8):
                nc.sync.dma_start(out=X[:, i, 0:512],
                                  in_=xt.ap()[ts(i, 128), 0:512])
        for i in range(8):
            nc.sync.dma_start(out=wsb["wv"][:, i, :],
                              in_=w_in["wv"].ap()[ts(i, 128), :])
        if first:
            for i in range(8):
                nc.sync.dma_start(out=X[:, i, 512:TPC],
                                  in_=xt.ap()[ts(i, 128), 512:TPC])
            for nm in ("wq", "wg", "wo"):
                for i in range(8):
                    nc.sync.dma_start(out=wsb[nm][:, i, :],
                                      in_=w_in[nm].ap()[ts(i, 128), :])
            bg_ap = bg_d.ap()
            nc.sync.dma_start(
                out=bg_sb[:],
                in_=bass.AP(tensor=bg_ap.tensor, offset=0,
                            ap=[[1, 128], [128, 8]]),
            )
            nc.sync.dma_start(out=sel[:], in_=sel_d.ap())

        # two 4-pair tiles: a [128,8,65] f32 tile (2080B/partition) would
        # straddle a PSUM bank boundary, which matmul writes cannot do
        kv_lo = kv_pool.tile([128, 4, 65], F32, tag="kv0", name="kv_lo",
                             padded_shape=[128, 4, 128])
        kv_hi = kv_pool.tile([128, 4, 65], F32, tag="kv1", name="kv_hi",
                             padded_shape=[128, 4, 128])
        kv_halves = (kv_lo, kv_hi)
        pend = []

        def _emit_kv(item):
            # start=True pending-zeroes the whole 2KB PSUM bank for the
            # instruction's partition range, so only the first region per
            # bank (j%4==0) may assert it; later regions' first writes land
            # on pending-zero bytes and accumulate from zero correctly.
            ksb, vp, mt = item
            for j in range(8):
                kvt = kv_halves[j // 4]
                for c in range(2):
                    h = 2 * j + c
                    nc.tensor.matmul(
                        kvt[64 * c:64 * c + 64, j % 4, :],
                        mm(ksb[:, ts(h, 64)]),
                        mm(vp[:, h, :]),
                        start=(mt == 0 and j % 4 == 0),
                        stop=(mt == NMT - 1),
                        skip_group_check=True,
                    )
        for mt in range(NMT):
            msl = ts(mt, 128)
            kps = pa_ps.tile([128, 1024], F32, tag="proj")
            for i in range(8):
                for o in range(2):
                    nc.tensor.matmul(
                        kps[:, ts(o, 512)],
                        mm(X[:, i, msl]),
                        mm(wsb["wk"][:, i, ts(o, 512)]),
                        start=(i == 0), stop=(i == 7),
                    )
            r1 = pa_tmp.tile([128, 1024], F32, tag="r1")
            nc.scalar.activation(r1, kps, AF.Relu)
            m1 = pa_tmp.tile([128, 1024], F32, tag="m1")
            nc.vector.tensor_scalar_min(m1, kps, 0.0)
            e1 = pa_tmp.tile([128, 1024], F32, tag="e1")
            nc.scalar.activation(e1, m1, AF.Exp)
            ksb = pa_tmp.tile([128, 1024], DT, tag="ksb", bufs=4)
            nc.gpsimd.tensor_add(ksb, r1, e1)

            vps = pa_ps.tile([128, 16, 64], F32, tag="proj")
            for i in range(8):
                for o in range(2):
                    nc.tensor.matmul(
                        vps[:, ts(o, 8), :],
                        mm(X[:, i, msl]),
                        mm(wsb["wv"][:, i, ts(o, 512)]),
                        start=(i == 0), stop=(i == 7),
                    )
            vp = pa_tmp.tile([128, 16, 65], DT, tag="vp", bufs=4)
            nc.vector.memset(vp[:, :, 64:65], 1.0)
            nc.scalar.copy(vp[:, :, 0:64], vps[:, :, :])

            pend.append((ksb, vp, mt))
            if mt > 1:
                _emit_kv(pend.pop(0))
        while pend:
            _emit_kv(pend.pop(0))
        kvs = pa_tmp.tile([128, 8, 65], DT, tag="kvs", bufs=1, name="kvs")
        nc.scalar.copy(kvs[:, 0:4, :], kv_lo[:])
        nc.scalar.copy(kvs[:, 4:8, :], kv_hi[:])
        nc.sync.dma_start(out=cc_in.ap()[:, :, :], in_=kvs[:])

    # ---------------- phase B ----------------
    with (
        tc.tile_pool(name="pb_tmp", bufs=2) as pb_tmp,
        tc.tile_pool(name="pb_small", bufs=1) as pb_small,
        tc.tile_pool(name="ps_misc", bufs=4, space="PSUM") as ps_misc,
        tc.tile_pool(name="ps_y", bufs=2, space="PSUM") as ps_y,
    ):
        kvf = pb_small.tile([128, 8, 65], DT, tag="kvf")
        # block-diagonal kv per head pair: [d(2 heads stacked), p, e(2 heads)]
        kvb = pb_small.tile([128, 8, 128], DT, tag="kvb")
        ksd = pb_small.tile([128, 8, 16], DT, tag="ksd")
        nc.vector.memset(kvb[:], 0.0)
        nc.vector.memset(ksd[:], 0.0)

        nc.gpsimd.collective_compute(
            "AllReduce",
            mybir.AluOpType.add,
            replica_groups=[[0, 1], [2, 3], [4, 5], [6, 7]],
            ins=[cc_in.ap().opt()],
            outs=[cc_out.ap().opt()],
        )

        def emit_kv_mat():
            # emitted AFTER two chunks of projections: these depend on the
            # collective, and the in-order DVE/scalar queues must not be
            # head-blocked on it while projection elementwise work waits
            nc.sync.dma_start(out=kvf[:], in_=cc_out.ap()[:, :, :])
            for p in range(8):
                nc.vector.tensor_copy(kvb[0:64, p, 0:64], kvf[0:64, p, 0:64])
                nc.vector.tensor_copy(kvb[64:128, p, 64:128],
                                      kvf[64:128, p, 0:64])
            for p in range(8):
                nc.scalar.activation(ksd[0:64, p, 2 * p:2 * p + 1],
                                     kvf[0:64, p, 64:65],
                                     AF.Copy, scale=1.0 / SCALE)
                nc.scalar.activation(ksd[64:128, p, 2 * p + 1:2 * p + 2],
                                     kvf[64:128, p, 64:65],
                                     AF.Copy, scale=1.0 / SCALE)

        qg = {}

        def emit_proj(ch, which="qg"):
            # chunks 0/1 are emitted before the collective completes; their
            # elu adds must stay off the Pool queue (the collective holds it)
            eng = nc.vector if ch < 2 else nc.gpsimd
            csl = ts(ch, CHUNK)
            if ch not in qg:
                qsb = pb_big.tile([128, 8, CHUNK], DT, tag="qsb",
                                  name=f"qsb{ch}")
                gsb = pb_big.tile([128, 8, CHUNK], DT, tag="gsb",
                                  name=f"gsb{ch}")
                qg[ch] = (qsb, gsb)
            qsb, gsb = qg[ch]
            if "g" not in which:
                gsb = None
            if "q" not in which:
                qsb = None
            # all Q then all G: exp and sigmoid live in different scalar
            # act tables; batching avoids per-head table reloads
            for p in range(8 if qsb is not None else 0):
                qps = ps_proj.tile([128, CHUNK], F32, tag="proj",
                                   name=f"qps{ch}_{p}")
                for i in range(8):
                    nc.tensor.matmul(
                        qps, mm(wsb["wq"][:, i, ts(p, 128)]),
                        mm(X[:, i, csl]),
                        start=(i == 0), stop=(i == 7),
                    )
                r1 = pb_tmp.tile([128, CHUNK], F32, tag="br1")
                nc.scalar.activation(r1, qps, AF.Relu)
                m1 = pb_tmp.tile([128, CHUNK], F32, tag="bm1")
                nc.vector.tensor_scalar_min(m1, qps, 0.0)
                e1 = pb_tmp.tile([128, CHUNK], F32, tag="be1")
                nc.scalar.activation(e1, m1, AF.Exp)
                eng.tensor_add(qsb[:, p, :], r1, e1)
            for p in range(8 if gsb is not None else 0):
                gps = ps_proj.tile([128, CHUNK], F32, tag="proj",
                                   name=f"gps{ch}_{p}")
                for i in range(8):
                    nc.tensor.matmul(
                        gps, mm(wsb["wg"][:, i, ts(p, 128)]),
                        mm(X[:, i, csl]),
                        start=(i == 0), stop=(i == 7),
                    )
                nc.scalar.activation(gsb[:, p, :], gps, AF.Sigmoid,
                                     bias=bg_sb[:, p:p + 1])

        zqrs = {}

        def emit_qk(ch):
            qsb, gsb = qg[ch]
            qkps = ps_misc.tile([16, CHUNK], F32, tag="misc",
                                name=f"qk{ch}")
            for p in range(8):
                nc.tensor.matmul(
                    qkps, mm(ksd[:, p, :]), mm(qsb[:, p, :]),
                    start=(p == 0), stop=(p == 7),
                    skip_group_check=True,
                )
            zq = pb_tmp.tile([16, CHUNK], F32, tag="zq")
            nc.vector.tensor_scalar_max(zq, qkps, CLAMP)
            zr = pb_tmp.tile([16, CHUNK], F32, tag="zr")
            nc.vector.reciprocal(zr, zq)
            zqr = pb_tmp.tile([16, CHUNK], DT, tag="zqr", bufs=2,
                              name=f"zqr{ch}")
            nc.vector.tensor_copy(zqr, zr)
            zqrs[ch] = zqr

        def emit_attn(ch, mid_cb=None):
            csl = ts(ch, CHUNK)
            qsb, gsb = qg.pop(ch)
            zqr = zqrs.pop(ch)
            asb = pb_big.tile([128, 8, CHUNK], DT, tag="asb",
                              name=f"asb{ch}")
            for p in range(8):
                zbps = ps_misc.tile([128, CHUNK], F32, tag="misc",
                                    name=f"zb{ch}_{p}")
                nc.tensor.matmul(zbps, mm(sel[:, p, :]), mm(zqr),
                                 start=True, stop=True)
                ops_ = ps_misc.tile([128, CHUNK], F32, tag="misc",
                                    name=f"op{ch}_{p}")
                nc.tensor.matmul(ops_, mm(kvb[:, p, :]), mm(qsb[:, p, :]),
                                 start=True, stop=True)
                t1 = pb_tmp.tile([128, CHUNK], F32, tag="bt1")
                # both muls read PSUM operands -> must be DVE (gpsimd
                # cannot access PSUM on hardware)
                nc.vector.tensor_mul(t1, ops_, gsb[:, p, :])
                nc.vector.tensor_mul(asb[:, p, :], t1, zbps)

            if mid_cb is not None:
                mid_cb()
            for d in range(8):
                yps = ps_y.tile([128, CHUNK], F32, tag="y",
                                name=f"y{ch}_{d}")
                for fi in range(8):
                    nc.tensor.matmul(
                        yps, mm(wsb["wo"][:, fi, ts(d, 128)]),
                        mm(asb[:, fi, :]),
                        start=(fi == 0), stop=(fi == 7),
                    )
                ysb = pb_tmp.tile([128, CHUNK], F32, tag="ysb")
                nc.scalar.copy(ysb, yps)
                nc.sync.dma_start(out=y_d.ap()[ts(d, 128), csl], in_=ysb[:])

        emit_proj(0)
        emit_proj(1)
        emit_kv_mat()
        emit_qk(0)
        emit_attn(0, mid_cb=lambda: emit_qk(1))
        emit_proj(2)
        emit_attn(1, mid_cb=lambda: emit_qk(2))
        emit_proj(3)
        emit_attn(2, mid_cb=lambda: emit_qk(3))
        emit_attn(3)


def _np_dt(dt_mode):
    return ml_dtypes.bfloat16 if dt_mode == "bf16" else np.float32


def prep_inputs(x, Wq, Wk, Wv, Wg, bg, Wo, dt_mode=DT_MODE):
    npdt = _np_dt(dt_mode)
    x_f = np.ascontiguousarray(np.asarray(x, np.float32).reshape(B * N, DIM))
    w_t = {}
    for nm, W in (("wq", Wq), ("wk", Wk), ("wv", Wv), ("wg", Wg)):
        w_t[nm] = np.ascontiguousarray(
            np.asarray(W, np.float32).T).astype(npdt)
    w_t["wo"] = np.ascontiguousarray(
        np.asarray(Wo, np.float32).T).astype(npdt)
    bg_f = np.ascontiguousarray(np.asarray(bg, np.float32))
    in_maps = []
    for c in range(N_CORES):
        xt_c = np.ascontiguousarray(
            x_f[c * TPC:(c + 1) * TPC].T).astype(npdt)
        m = {"xt": xt_c, "bg": bg_f}
        m.update(w_t)
        in_maps.append(m)
    return in_maps


def unshard_output(y_parts):
    out = np.empty((B * N, DIM), np.float32)
    for c in range(N_CORES):
        out[c * TPC:(c + 1) * TPC] = np.asarray(y_parts[c]).T
    return out.reshape(B, N, DIM)


def get_nc(dt_mode=DT_MODE):
    key = ("nc", dt_mode)
    if key not in _CACHE:
        _CACHE[key] = _build(dt_mode)
    return _CACHE[key]


def kernel(x, Wq, Wk, Wv, Wg, bg, Wo):
    from concourse.bass_utils import run_bass_kernel_spmd

    nc = get_nc()
    in_maps = prep_inputs(x, Wq, Wk, Wv, Wg, bg, Wo)
    res = run_bass_kernel_spmd(nc, in_maps, core_ids=list(range(N_CORES)))
    return unshard_output([res.results[c]["y"] for c in range(N_CORES)])


# revision 20
# speedup vs baseline: 1.0284x; 1.0284x over previous
"""Trainium2 Bass kernel for nn_GatedAttention (linear attention with sigmoid
gate).

Strategy: shard the 16384 token rows across 8 cores (2048 each; cores 2b,2b+1
hold batch b). Per core, two phases:
  A: K,V projections (token-major) + per-head kv' = K^T [V|1] accumulated in
     PSUM over all local tokens (the ones column folds k_sum into kv').
  -- pairwise AllReduce of kv' between the two cores sharing a batch --
  B: Q,G projections (feature-major), out^T = kv'^T @ Q per head (block-diag
     per head pair), normalizer z = SCALE/max(q.k_sum,eps) applied via tiny
     selector matmuls, gate, and the final output projection, feature-major.
Host transposes x to feature-major and pre-transposes weights; output returns
feature-major per-core slabs that the host transposes back.

Scheduling notes (sim-derived):
 - DMA order: wk, X[:, :512], wv, X[:, 512:], wq, wg, wo -- the first K-proj
   matmul only waits for ~3MB instead of the full ~14MB input set.
 - kv' accumulates in PSUM across all 16 m-tiles (start/stop at the ends),
   no per-mtile DVE adds.
 - ps_proj PSUM pool is allocated before phase A's pools so phase B's Q/G
   projections don't wait on a PSUM bank recycle barrier.
 - Phase B emits projections one chunk ahead of attention tails so the
   AllReduce hides behind ~54us of independent matmuls.
 - Q-elu and G-sigmoid are batched (8+8, not interleaved): exp and sigmoid
   live in different scalar-engine act tables; interleaving reloads ~1.3us.
 - y stores DMA directly from PSUM (no scalar copy).
"""
import sys

sys.path.insert(0, "/opt/trn_rl_repo")

import numpy as np
import ml_dtypes

B, N, DIM = 4, 4096, 1024
HEADS, DH = 16, 64
SCALE = DH ** -0.5
N_CORES = 8
TPC = B * N // N_CORES      # 2048 tokens per core
NMT = TPC // 128            # 16 m-tiles (phase A)
CHUNK = 512
NCH = TPC // CHUNK          # 4 chunks (phase B)
CLAMP = 1e-6 / SCALE

DT_MODE = "bf16"            # "bf16" | "f32r" | "f32"

_CACHE = {}


def _build(dt_mode=DT_MODE, reps=1):
    import concourse.bacc as bacc
    import concourse.bass as bass
    import concourse.tile as tile
    from concourse import mybir

    AF = mybir.ActivationFunctionType
    F32 = mybir.dt.float32
    DT = mybir.dt.bfloat16 if dt_mode == "bf16" else mybir.dt.float32

    def mm(ap):
        # matmul-operand view: reduced-precision f32 mode uses float32r APs
        return ap.bitcast(mybir.dt.float32r) if dt_mode == "f32r" else ap

    ts = bass.ts

    nc = bacc.Bacc("TRN2", target_bir_lowering=False, debug=False,
                   num_devices=N_CORES)
    xt = nc.dram_tensor("xt", [DIM, TPC], DT, kind="ExternalInput")
    w_in = {}
    for nm in ("wk", "wv", "wq", "wg", "wo"):
        w_in[nm] = nc.dram_tensor(nm, [DIM, DIM], DT, kind="ExternalInput")
    bg_d = nc.dram_tensor("bg", [DIM], F32, kind="ExternalInput")
    y_d = nc.dram_tensor("y", [DIM, TPC], F32, kind="ExternalOutput")
    cc_in = nc.dram_tensor("cc_in", [128, 8, 65], DT)
    cc_out = nc.dram_tensor("cc_out", [128, 8, 65], DT)

    with tile.TileContext(nc, num_cores=N_CORES) as tc:
        with (
            tc.tile_pool(name="persist", bufs=1) as persist,
            tc.tile_pool(name="pb_big", bufs=2) as pb_big,
            tc.tile_pool(name="ps_proj", bufs=2, space="PSUM") as ps_proj,
        ):
            X = persist.tile([128, 8, TPC], DT, tag="x")
            wsb = {}
            for nm in ("wq", "wg", "wo"):
                wsb[nm] = persist.tile([128, 8, DIM], DT, tag=nm, name=nm)
            bg_sb = persist.tile([128, 8], F32, tag="bg")
            sel_np = np.zeros((16, 8, 128), _np_dt(dt_mode))
            for p in range(8):
                sel_np[2 * p, p, 0:64] = 1.0
                sel_np[2 * p + 1, p, 64:128] = 1.0
            sel_d = nc.inline_tensor(sel_np, name="sel_const")
            sel = persist.tile([16, 8, 128], DT, tag="sel")

            for _rep in range(reps):
                _phases(nc, tc, bass, mybir, AF, F32, DT, mm, ts, X, wsb,
                        bg_sb, sel, sel_d, xt, bg_d, w_in, cc_in, cc_out,
                        y_d, ps_proj, pb_big, first=(_rep == 0))
    nc.compile()
    return nc


def _phases(nc, tc, bass, mybir, AF, F32, DT, mm, ts, X, wsb, bg_sb, sel,
            sel_d, xt, bg_d, w_in, cc_in, cc_out, y_d, ps_proj, pb_big,
            first):
    # ---------------- phase A ----------------
    with (
        tc.tile_pool(name="pa_w", bufs=1) as pa_w,
        tc.tile_pool(name="pa_tmp", bufs=2) as pa_tmp,
        tc.tile_pool(name="pa_ps", bufs=2, space="PSUM") as pa_ps,
        tc.tile_pool(name="kv_ps", bufs=1, space="PSUM") as kv_pool,
    ):
        for nm in ("wk", "wv"):
            wsb[nm] = pa_w.tile([128, 8, DIM], DT, tag=nm, name=nm)
        # DMA issue order gates PE start: wk then the first 512 token
        # columns of X unblock the first K-proj after ~3MB of traffic.
        for i in range(8):
            nc.sync.dma_start(out=wsb["wk"][:, i, :],
                              in_=w_in["wk"].ap()[ts(i, 128), :])
        if first:
            for i in range(8):
                nc.sync.dma_start(out=X[:, i, 0:512],
                                  in_=xt.ap()[ts(i, 128), 0:512])
        for i in range(8):
            nc.sync.dma_start(out=wsb["wv"][:, i, :],
                              in_=w_in["wv"].ap()[ts(i, 128), :])
        if first:
            for i in range(8):
                nc.sync.dma_start(out=X[:, i, 512:TPC],
                                  in_=xt.ap()[ts(i, 128), 512:TPC])
            for nm in ("wq", "wg", "wo"):
                for i in range(8):
                    nc.sync.dma_start(out=wsb[nm][:, i, :],
                                      in_=w_in[nm].ap()[ts(i, 128), :])
            bg_ap = bg_d.ap()
            nc.sync.dma_start(
                out=bg_sb[:],
                in_=bass.AP(tensor=bg_ap.tensor, offset=0,
                            ap=[[1, 128], [128, 8]]),
            )
            nc.sync.dma_start(out=sel[:], in_=sel_d.ap())

        # two 4-pair tiles: a [128,8,65] f32 tile (2080B/partition) would
        # straddle a PSUM bank boundary, which matmul writes cannot do
        kv_lo = kv_pool.tile([128, 4, 65], F32, tag="kv0", name="kv_lo",
                             padded_shape=[128, 4, 128])
        kv_hi = kv_pool.tile([128, 4, 65], F32, tag="kv1", name="kv_hi",
                             padded_shape=[128, 4, 128])
        kv_halves = (kv_lo, kv_hi)
        pend = []

        def _emit_kv(item):
            # start=True pending-zeroes the whole 2KB PSUM bank for the
            # instruction's partition range, so only the first region per
            # bank (j%4==0) may assert it; later regions' first writes land
            # on pending-zero bytes and accumulate from zero correctly.
            ksb, vp, mt = item
            for j in range(8):
                kvt = kv_halves[j // 4]
                for c in range(2):
                    h = 2 * j + c
                    nc.tensor.matmul(
                        kvt[64 * c:64 * c + 64, j % 4, :],
                        mm(ksb[:, ts(h, 64)]),
                        mm(vp[:, h, :]),
                        start=(mt == 0 and j % 4 == 0),
                        stop=(mt == NMT - 1),
                        skip_group_check=True,
                    )
        for mt in range(NMT):
            msl = ts(mt, 128)
            kps = pa_ps.tile([128, 1024], F32, tag="proj")
            for i in range(8):
                for o in range(2):
                    nc.tensor.matmul(
                        kps[:, ts(o, 512)],
                        mm(X[:, i, msl]),
                        mm(wsb["wk"][:, i, ts(o, 512)]),
                        start=(i == 0), stop=(i == 7),
                    )
            r1 = pa_tmp.tile([128, 1024], DT, tag="r1")
            nc.scalar.activation(r1, kps, AF.Relu)
            m1 = pa_tmp.tile([128, 1024], DT, tag="m1")
            nc.vector.tensor_scalar_min(m1, kps, 0.0)
            e1 = pa_tmp.tile([128, 1024], DT, tag="e1")
            nc.scalar.activation(e1, m1, AF.Exp)
            ksb = pa_tmp.tile([128, 1024], DT, tag="ksb", bufs=4)
            nc.gpsimd.tensor_add(ksb, r1, e1)

            vps = pa_ps.tile([128, 16, 64], F32, tag="proj")
            for i in range(8):
                for o in range(2):
                    nc.tensor.matmul(
                        vps[:, ts(o, 8), :],
                        mm(X[:, i, msl]),
                        mm(wsb["wv"][:, i, ts(o, 512)]),
                        start=(i == 0), stop=(i == 7),
                    )
            vp = pa_tmp.tile([128, 16, 65], DT, tag="vp", bufs=4)
            nc.vector.memset(vp[:, :, 64:65], 1.0)
            nc.scalar.copy(vp[:, :, 0:64], vps[:, :, :])

            pend.append((ksb, vp, mt))
            if mt > 1:
                _emit_kv(pend.pop(0))
        while pend:
            _emit_kv(pend.pop(0))
        kvs = pa_tmp.tile([128, 8, 65], DT, tag="kvs", bufs=1, name="kvs")
        nc.scalar.copy(kvs[:, 0:4, :], kv_lo[:])
        nc.scalar.copy(kvs[:, 4:8, :], kv_hi[:])
        nc.sync.dma_start(out=cc_in.ap()[:, :, :], in_=kvs[:])

    # ---------------- phase B ----------------
    with (
        tc.tile_pool(name="pb_tmp", bufs=2) as pb_tmp,
        tc.tile_pool(name="pb_small", bufs=1) as pb_small,
        tc.tile_pool(name="ps_misc", bufs=4, space="PSUM") as ps_misc,
        tc.tile_pool(name="ps_y", bufs=2, space="PSUM") as ps_y,
    ):
        kvf = pb_small.tile([128, 8, 65], DT, tag="kvf")
        # block-diagonal kv per head pair: [d(2 heads stacked), p, e(2 heads)]
        kvb = pb_small.tile([128, 8, 128], DT, tag="kvb")
        ksd = pb_small.tile([128, 8, 16], DT, tag="ksd")
        nc.vector.memset(kvb[:], 0.0)
        nc.vector.memset(ksd[:], 0.0)

        nc.gpsimd.collective_compute(
            "AllReduce",
            mybir.AluOpType.add,
            replica_groups=[[0, 1], [2, 3], [4, 5], [6, 7]],
            ins=[cc_in.ap().opt()],
            outs=[cc_out.ap().opt()],
        )

        def emit_kv_mat():
            # emitted AFTER two chunks of projections: these depend on the
            # collective, and the in-order DVE/scalar queues must not be
            # head-blocked on it while projection elementwise work waits
            nc.sync.dma_start(out=kvf[:], in_=cc_out.ap()[:, :, :])
            for p in range(8):
                nc.vector.tensor_copy(kvb[0:64, p, 0:64], kvf[0:64, p, 0:64])
                nc.vector.tensor_copy(kvb[64:128, p, 64:128],
                                      kvf[64:128, p, 0:64])
            for p in range(8):
                nc.scalar.activation(ksd[0:64, p, 2 * p:2 * p + 1],
                                     kvf[0:64, p, 64:65],
                                     AF.Copy, scale=1.0 / SCALE)
                nc.scalar.activation(ksd[64:128, p, 2 * p + 1:2 * p + 2],
                                     kvf[64:128, p, 64:65],
                                     AF.Copy, scale=1.0 / SCALE)

        qg = {}

        def emit_proj(ch, which="qg"):
            # chunks 0/1 are emitted before the collective completes; their
            # elu adds must stay off the Pool queue (the collective holds it)
            eng = nc.vector if ch < 2 else nc.gpsimd
            csl = ts(ch, CHUNK)
            if ch not in qg:
                qsb = pb_big.tile([128, 8, CHUNK], DT, tag="qsb", bufs=3,
                                  name=f"qsb{ch}")
                gsb = pb_big.tile([128, 8, CHUNK], DT, tag="gsb", bufs=3,
                                  name=f"gsb{ch}")
                qg[ch] = (qsb, gsb)
            qsb, gsb = qg[ch]
            if "g" not in which:
                gsb = None
            if "q" not in which:
                qsb = None
            # all Q then all G: exp and sigmoid live in different scalar
            # act tables; batching avoids per-head table reloads
            for p in range(8 if qsb is not None else 0):
                qps = ps_proj.tile([128, CHUNK], F32, tag="proj",
                                   name=f"qps{ch}_{p}")
                for i in range(8):
                    nc.tensor.matmul(
                        qps, mm(wsb["wq"][:, i, ts(p, 128)]),
                        mm(X[:, i, csl]),
                        start=(i == 0), stop=(i == 7),
                    )
                r1 = pb_tmp.tile([128, CHUNK], DT, tag="br1")
                nc.scalar.activation(r1, qps, AF.Relu)
                m1 = pb_tmp.tile([128, CHUNK], DT, tag="bm1")
                nc.vector.tensor_scalar_min(m1, qps, 0.0)
                e1 = pb_tmp.tile([128, CHUNK], DT, tag="be1")
                nc.scalar.activation(e1, m1, AF.Exp)
                eng.tensor_add(qsb[:, p, :], r1, e1)
            for p in range(8 if gsb is not None else 0):
                gps = ps_proj.tile([128, CHUNK], F32, tag="proj",
                                   name=f"gps{ch}_{p}")
                for i in range(8):
                    nc.tensor.matmul(
                        gps, mm(wsb["wg"][:, i, ts(p, 128)]),
                        mm(X[:, i, csl]),
                        start=(i == 0), stop=(i == 7),
                    )
                nc.scalar.activation(gsb[:, p, :], gps, AF.Sigmoid,
                                     bias=bg_sb[:, p:p + 1])

        zqrs = {}

        def emit_qk(ch):
            qsb, gsb = qg[ch]
            qkps = ps_misc.tile([16, CHUNK], F32, tag="misc",
                                name=f"qk{ch}")
            for p in range(8):
                nc.tensor.matmul(
                    qkps, mm(ksd[:, p, :]), mm(qsb[:, p, :]),
                    start=(p == 0), stop=(p == 7),
                    skip_group_check=True,
                )
            zq = pb_tmp.tile([16, CHUNK], F32, tag="zq")
            nc.vector.tensor_scalar_max(zq, qkps, CLAMP)
            zr = pb_tmp.tile([16, CHUNK], F32, tag="zr")
            nc.vector.reciprocal(zr, zq)
            zqr = pb_tmp.tile([16, CHUNK], DT, tag="zqr", bufs=2,
                              name=f"zqr{ch}")
            nc.vector.tensor_copy(zqr, zr)
            zqrs[ch] = zqr

        def emit_attn(ch, mid_cb=None):
            csl = ts(ch, CHUNK)
            qsb, gsb = qg.pop(ch)
            zqr = zqrs.pop(ch)
            asb = pb_big.tile([128, 8, CHUNK], DT, tag="asb",
                              name=f"asb{ch}")
            for p in range(8):
                zbps = ps_misc.tile([128, CHUNK], F32, tag="misc",
                                    name=f"zb{ch}_{p}")
                nc.tensor.matmul(zbps, mm(sel[:, p, :]), mm(zqr),
                                 start=True, stop=True)
                ops_ = ps_misc.tile([128, CHUNK], F32, tag="misc",
                                    name=f"op{ch}_{p}")
                nc.tensor.matmul(ops_, mm(kvb[:, p, :]), mm(qsb[:, p, :]),
                                 start=True, stop=True)
                t1 = pb_tmp.tile([128, CHUNK], F32, tag="bt1")
                # both muls read PSUM operands -> must be DVE (gpsimd
                # cannot access PSUM on hardware)
                nc.vector.tensor_mul(t1, ops_, gsb[:, p, :])
                nc.vector.tensor_mul(asb[:, p, :], t1, zbps)

            if mid_cb is not None:
                mid_cb()
            for d in range(8):
                yps = ps_y.tile([128, CHUNK], F32, tag="y",
                                name=f"y{ch}_{d}")
                for fi in range(8):
                    nc.tensor.matmul(
                        yps, mm(wsb["wo"][:, fi, ts(d, 128)]),
                        mm(asb[:, fi, :]),
                        start=(fi == 0), stop=(fi == 7),
                    )
                ysb = pb_tmp.tile([128, CHUNK], F32, tag="ysb")
                nc.scalar.copy(ysb, yps)
                nc.sync.dma_start(out=y_d.ap()[ts(d, 128), csl], in_=ysb[:])

        emit_proj(0)
        emit_proj(1)
        emit_proj(2)
        emit_kv_mat()
        emit_qk(0)
        emit_attn(0, mid_cb=lambda: emit_qk(1))
        emit_attn(1, mid_cb=lambda: emit_qk(2))
        emit_proj(3)
        emit_attn(2, mid_cb=lambda: emit_qk(3))
        emit_attn(3)


def _np_dt(dt_mode):
    return ml_dtypes.bfloat16 if dt_mode == "bf16" else np.float32


def prep_inputs(x, Wq, Wk, Wv, Wg, bg, Wo, dt_mode=DT_MODE):
    npdt = _np_dt(dt_mode)
    x_f = np.ascontiguousarray(np.asarray(x, np.float32).reshape(B * N, DIM))
    w_t = {}
    for nm, W in (("wq", Wq), ("wk", Wk), ("wv", Wv), ("wg", Wg)):
        w_t[nm] = np.ascontiguousarray(
            np.asarray(W, np.float32).T).astype(npdt)
    w_t["wo"] = np.ascontiguousarray(
        np.asarray(Wo, np.float32).T).astype(npdt)
    bg_f = np.ascontiguousarray(np.asarray(bg, np.float32))
    in_maps = []
    for c in range(N_CORES):
        xt_c = np.ascontiguousarray(
            x_f[c * TPC:(c + 1) * TPC].T).astype(npdt)
        m = {"xt": xt_c, "bg": bg_f}
        m.update(w_t)
        in_maps.append(m)
    return in_maps


def unshard_output(y_parts):
    out = np.empty((B * N, DIM), np.float32)
    for c in range(N_CORES):
        out[c * TPC:(c + 1) * TPC] = np.asarray(y_parts[c]).T
    return out.reshape(B, N, DIM)


def get_nc(dt_mode=DT_MODE):
    key = ("nc", dt_mode)
    if key not in _CACHE:
        _CACHE[key] = _build(dt_mode)
    return _CACHE[key]


def kernel(x, Wq, Wk, Wv, Wg, bg, Wo):
    from concourse.bass_utils import run_bass_kernel_spmd

    nc = get_nc()
    in_maps = prep_inputs(x, Wq, Wk, Wv, Wg, bg, Wo)
    res = run_bass_kernel_spmd(nc, in_maps, core_ids=list(range(N_CORES)))
    return unshard_output([res.results[c]["y"] for c in range(N_CORES)])
